# revision 29
# baseline (speedup 1.0000x reference)
"""Tied-row (MSA) attention, sharded over 8 TRN2 NeuronCores.

Reference computation (b=1, r=128 MSA rows, n=512, 8 heads x 64):
    q, k, v = x @ Wq, x @ Wk, x @ Wv          per-row projections
    dots[h,i,j] = sum_{r,d} q[r,h,i,d] k[r,h,j,d] * scale / sqrt(num_rows)
    attn = softmax_j(dots)                     shared across rows
    out[r,i] = (sum_j attn[h,i,j] v[r,h,j,d]) @ Wo + bo

Sharding: MSA-row axis r split 16-per-core.  Each core computes its partial
logits (reduction over its local r); partials are summed with one bf16
AllReduce per head-pair, pipelined behind the following pairs' matmuls.
Every core softmaxes all heads locally (replicated work, no second
collective).  A tiny warmup AllReduce fires first so the cross-core
start-stagger / ncfw cold-start is absorbed before real data is ready.

Host-side prep: x is pre-transposed and pre-cast to bf16 x^T per core and
the projection weights are pre-arranged/pre-cast, so the device kernel
starts matmuls as soon as the first x^T chunk lands.  The output is
returned dim-major bf16 (out^T) and de-transposed on the host.
"""

import numpy as np
import ml_dtypes

import concourse.bacc as bacc
import concourse.bass as bass
import concourse.mybir as mybir
import concourse.tile as tile
from concourse import bass_utils
from concourse.masks import make_identity

CORES = 8
R = 16          # MSA rows per core
N = 512         # sequence length
DIM = 256       # model dim
H = 8           # heads
D = 64          # head dim
HD = H * D      # 512
RN = R * N      # 8192 token-rows per core

F32 = mybir.dt.float32
BF16 = mybir.dt.bfloat16

RG = [list(range(CORES))]


def build_nc(scale: float):
    nc = bacc.Bacc(None, target_bir_lowering=False, debug=False)

    xt_ext = nc.declare_dram_parameter("xt", [128, 2, RN], BF16, isOutput=False)
    wq_ext = nc.declare_dram_parameter("wq", [128, 2, HD], BF16, isOutput=False)
    wk_ext = nc.declare_dram_parameter("wk", [128, 2, HD], BF16, isOutput=False)
    wv_ext = nc.declare_dram_parameter("wv", [128, 2, HD], BF16, isOutput=False)
    wo_ext = nc.declare_dram_parameter("wo", [128, 4, DIM], BF16, isOutput=False)
    outT_ext = nc.declare_dram_parameter("outT", [2, 128, RN], BF16, isOutput=True)

    # alternate PSUM->SBUF copies between DVE and ScalarE so neither gates
    # PSUM-bank recycling (phase-1 only; the back phase pins engines)
    _cp = [0]

    def cp(out, in_):
        if _cp[0] % 2 == 0:
            nc.vector.tensor_copy(out, in_)
        else:
            nc.scalar.copy(out, in_)
        _cp[0] += 1

    with tile.TileContext(nc) as tc:
        # ---- DRAM bounce buffers: one AllReduce per head-pair ----
        dram = tc.alloc_tile_pool(name="dram", bufs=1, space="DRAM")
        ar_in = [dram.tile([2 * N, N], BF16, tag=f"ar_in{hp}", name=f"ar_in{hp}") for hp in range(4)]
        wu_in = dram.tile([128, 8], BF16, tag="wu_in", name="wu_in")
        wu_out = dram.tile([128, 8], BF16, tag="wu_out", name="wu_out", addr_space="Shared")
        ar_out = [
            dram.tile([2 * N, N], BF16, tag=f"ar_out{hp}", name=f"ar_out{hp}", addr_space="Shared")
            for hp in range(4)
        ]

        # ---- SBUF pools (releases must be LIFO) ----
        consts = tc.alloc_tile_pool(name="consts", bufs=1)
        xT_pool = tc.alloc_tile_pool(name="xT", bufs=1)
        v_pool = tc.alloc_tile_pool(name="v", bufs=R * 4)
        attnT_pool = tc.alloc_tile_pool(name="attnT", bufs=1)
        qkT_pool = tc.alloc_tile_pool(name="qkT", bufs=1)
        dstage_pool = tc.alloc_tile_pool(name="dstage", bufs=4)

        # warmup AllReduce: the very first thing the kernel does, so the
        # cross-core launch stagger is absorbed here, not in AllReduce #0
        wu_sb = consts.tile([128, 8], BF16, tag="wu")
        nc.vector.memset(wu_sb[:], 0.0)
        nc.sync.dma_start(out=wu_in[:, :], in_=wu_sb[:])
        nc.gpsimd.collective_compute(
            "AllReduce",
            mybir.AluOpType.add,
            replica_groups=RG,
            ins=[wu_in[:, :].opt()],
            outs=[wu_out[:, :].opt()],
        )

        # weights first (first proj matmul needs wq), then x^T in 16 chunks
        # spread across all DMA queues so proj of chunk c starts early
        wq_sb = consts.tile([128, 2, HD], BF16, tag="wq")
        wk_sb = consts.tile([128, 2, HD], BF16, tag="wk")
        wv_sb = consts.tile([128, 2, HD], BF16, tag="wv")
        wo_sb = consts.tile([128, 4, DIM], BF16, tag="wo")
        idbf = consts.tile([128, 128], BF16, tag="idbf")
        for kc in range(2):
            nc.sync.dma_start(out=wq_sb[:, kc, :], in_=wq_ext[:, kc, :])
            nc.sync.dma_start(out=wk_sb[:, kc, :], in_=wk_ext[:, kc, :])

        xT = xT_pool.tile([128, 2, RN], BF16, tag="xT")
        for qc in range(16):
            nc.sync.dma_start(
                out=xT[:, :, qc * N:(qc + 1) * N],
                in_=xt_ext[:, :, qc * N:(qc + 1) * N],
            )

        nc.sync.dma_start(out=wv_sb[:], in_=wv_ext[:])
        nc.sync.dma_start(out=wo_sb[:], in_=wo_ext[:])
        make_identity(nc, idbf[:])

        # keep the PE busy while the first x^T chunks stream in, so the HAM
        # clock gate is already released (2.4 GHz) when the real matmuls start
        warm_psum = tc.alloc_tile_pool(name="warm_psum", bufs=1, space="PSUM")
        wt = warm_psum.tile([128, 128], BF16, tag="wt")
        for _ in range(96):
            nc.tensor.transpose(wt[:], idbf[:], idbf[:])
        warm_psum.release()

        proj_psum = tc.alloc_tile_pool(name="proj_psum", bufs=4, space="PSUM")
        dots_psum = tc.alloc_tile_pool(name="dots_psum", bufs=4, space="PSUM")

        attnT = attnT_pool.tile([128, H, 4, N], BF16, tag="attnT")

        # ---- per head-pair: q/k projection, partial dots, AllReduce ----
        for hp in range(4):
            qT = qkT_pool.tile([128, RN], BF16, tag="qT")
            kT = qkT_pool.tile([128, RN], BF16, tag="kT")
            for wsb, dstT in ((wq_sb, qT), (wk_sb, kT)):
                # groups of 4 chunks share one LDWEIGHTS per kc
                for g0 in range(0, RN // N, 4):
                    chs = range(g0, min(g0 + 4, RN // N))
                    pss = {
                        ch: proj_psum.tile([128, N], F32, tag="proj", name=f"proj{ch}")
                        for ch in chs
                    }
                    for kc in range(2):
                        for ch in chs:
                            nc.tensor.matmul(
                                pss[ch][:],
                                wsb[:, kc, hp * 128:(hp + 1) * 128],
                                xT[:, kc, ch * N:(ch + 1) * N],
                                start=(kc == 0),
                                stop=(kc == 1),
                            )
                    for ch in chs:
                        cp(dstT[:, ch * N:(ch + 1) * N], pss[ch][:])

            # partial dots for the two heads of this pair; the even head uses
            # PE row-group 0-63, the odd head 64-127 (concurrent row tiles)
            for ic in range(4):
                pe_ = dots_psum.tile([128, N], F32, tag="dots")
                po_ = dots_psum.tile([128, N], F32, tag="dots")
                for rr in range(R):
                    base = rr * N
                    isl = slice(base + ic * 128, base + ic * 128 + 128)
                    jsl = slice(base, base + N)
                    nc.tensor.matmul(
                        pe_[:],
                        qT[0:64, isl],
                        kT[0:64, jsl],
                        start=(rr == 0),
                        stop=(rr == R - 1),
                        skip_group_check=True,
                    )
                    nc.tensor.matmul(
                        po_[:],
                        qT[64:128, isl],
                        kT[64:128, jsl],
                        start=(rr == 0),
                        stop=(rr == R - 1),
                        skip_group_check=True,
                    )
                for m, ps in ((0, pe_), (1, po_)):
                    st = dstage_pool.tile([128, N], BF16, tag="dstage")
                    cp(st[:], ps[:])
                    row0 = m * N + ic * 128
                    nc.sync.dma_start(out=ar_in[hp][row0:row0 + 128, :], in_=st[:])

            nc.gpsimd.collective_compute(
                "AllReduce",
                mybir.AluOpType.add,
                replica_groups=RG,
                ins=[ar_in[hp][:, :].opt()],
                outs=[ar_out[hp][:, :].opt()],
            )

        dstage_pool.release()
        qkT_pool.release()
        dots_psum.release()

        # ---- v projection (overlaps the AllReduces; reads xT) ----
        # no AR-gated ops are enqueued on DVE/ScalarE during this phase, so
        # the PSUM drains can't get stuck behind a softmax EXP
        v_tiles = {}
        for rr in range(R):
            for jc in range(4):
                ps = proj_psum.tile([128, N], F32, tag="proj")
                for kc in range(2):
                    nc.tensor.matmul(
                        ps[:],
                        xT[:, kc, rr * N + jc * 128:rr * N + jc * 128 + 128],
                        wv_sb[:, kc, :],
                        start=(kc == 0),
                        stop=(kc == 1),
                    )
                vt = v_pool.tile([128, HD], BF16, tag="v")
                cp(vt[:], ps[:])
                v_tiles[(rr, jc)] = vt

        proj_psum.release()

        smax_pool = tc.alloc_tile_pool(name="smax", bufs=2)
        oT_pool = tc.alloc_tile_pool(name="oT", bufs=24)
        ostage_pool = tc.alloc_tile_pool(name="ostage", bufs=4)
        atp_psum = tc.alloc_tile_pool(name="atp_psum", bufs=2, space="PSUM")
        av_psum = tc.alloc_tile_pool(name="av_psum", bufs=2, space="PSUM")
        fin_psum = tc.alloc_tile_pool(name="fin_psum", bufs=4, space="PSUM")

        def smax(hp):
            """softmax both heads of AllReduce #hp, transpose into
            attnT[:, 2hp:2hp+2, :, :].  EXP on ScalarE, normalize on GpSimd,
            PSUM drains on DVE - issue only where AR #hp is surely done."""
            for m in range(2):
                h = 2 * hp + m
                abfs = []
                for ic in range(4):
                    zt = smax_pool.tile([128, N], BF16, tag="zt", bufs=3)
                    row0 = m * N + ic * 128
                    nc.sync.dma_start(out=zt[:], in_=ar_out[hp][row0:row0 + 128, :])
                    att_f = smax_pool.tile([128, N], F32, tag="att_f", bufs=3)
                    sums = smax_pool.tile([128, 1], F32, tag="sums", bufs=3)
                    nc.scalar.activation(
                        att_f[:],
                        zt[:],
                        mybir.ActivationFunctionType.Exp,
                        scale=scale,
                        accum_out=sums[:],
                    )
                    recip = smax_pool.tile([128, 1], F32, tag="recip", bufs=3)
                    nc.vector.reciprocal(recip[:], sums[:])
                    abf = smax_pool.tile([128, N], BF16, tag="abf", bufs=8)
                    nc.vector.tensor_scalar_mul(abf[:], att_f[:], recip[:])
                    abfs.append(abf)
                for jt in range(4):
                    pt = atp_psum.tile([128, N], BF16, tag="atp")
                    for ic in range(4):
                        nc.tensor.transpose(
                            pt[:, ic * 128:(ic + 1) * 128],
                            abfs[ic][:, jt * 128:(jt + 1) * 128],
                            idbf[:],
                        )
                    nc.vector.tensor_copy(attnT[:, h, jt, :], pt[:])

        smax(0)

        # ---- attn^T @ v -> out^T (bf16), then Wo^T-projected out^T ----
        # r processed in quarters: all four head-pair blocks for 4 rows, then
        # their output projection; softmax #2/#3 are issued inside quarter 0
        # so their AllReduces are done by the time ScalarE reaches the EXPs
        for rq in range(4):
            oTs = {}
            for hp in range(4):
                if rq == 0 and hp >= 1:
                    smax(hp)
                for rx in range(4):
                    rr = rq * 4 + rx
                    ps = av_psum.tile([128, N], F32, tag="av")
                    for jt in range(4):
                        for m in range(2):
                            h = 2 * hp + m
                            nc.tensor.matmul(
                                ps[m * 64:(m + 1) * 64, :],
                                v_tiles[(rr, jt)][:, h * D:(h + 1) * D],
                                attnT[:, h, jt, :],
                                start=(jt == 0),
                                stop=(jt == 3),
                                tile_position=(0, m * 64),
                                skip_group_check=True,
                            )
                    oT = oT_pool.tile([128, N], BF16, tag="oT")
                    nc.vector.tensor_copy(oT[:], ps[:])
                    oTs[(rx, hp)] = oT
            # Wo stationary is reused across the quarter's 4 rows: kc-outer,
            # rx-inner, with 4 accumulators open per dim-chunk
            for dc in range(2):
                psfs = {
                    rx: fin_psum.tile([128, N], F32, tag="fin", name=f"fin{rx}")
                    for rx in range(4)
                }
                for kc in range(4):
                    for rx in range(4):
                        nc.tensor.matmul(
                            psfs[rx][:],
                            wo_sb[:, kc, dc * 128:(dc + 1) * 128],
                            oTs[(rx, kc)][:],
                            start=(kc == 0),
                            stop=(kc == 3),
                        )
                for rx in range(4):
                    rr = rq * 4 + rx
                    ost = ostage_pool.tile([128, N], BF16, tag="ost")
                    nc.scalar.copy(ost[:], psfs[rx][:])
                    nc.sync.dma_start(
                        out=outT_ext[dc, :, rr * N:(rr + 1) * N], in_=ost[:]
                    )

        fin_psum.release()
        av_psum.release()
        atp_psum.release()
        ostage_pool.release()
        oT_pool.release()
        smax_pool.release()
        attnT_pool.release()
        v_pool.release()
        xT_pool.release()
        consts.release()
        dram.release()

    if not nc.is_finalized():
        nc.finalize()
    return nc


_cache = {}


def _get_nc(scale: float):
    key = round(float(scale), 12)
    if key not in _cache:
        _cache[key] = build_nc(float(scale))
    return _cache[key]


def make_in_maps(x, Wq, Wkv, Wo):
    bf16 = ml_dtypes.bfloat16
    x = np.asarray(x, dtype=np.float32).reshape(CORES, RN, DIM)
    # per-core x^T, laid out [128 partitions, 2 dim-chunks, RN tokens]
    xts = [
        np.ascontiguousarray(
            x[c].T.reshape(2, 128, RN).transpose(1, 0, 2)
        ).astype(bf16)
        for c in range(CORES)
    ]
    Wq = np.asarray(Wq, dtype=np.float32)
    Wkv = np.asarray(Wkv, dtype=np.float32)
    Wo = np.asarray(Wo, dtype=np.float32)

    def warr(w, kchunks):  # [kchunks*128, n] -> [128, kchunks, n] bf16
        n = w.shape[1]
        return np.ascontiguousarray(
            w.reshape(kchunks, 128, n).transpose(1, 0, 2)
        ).astype(bf16)

    wq = warr(Wq, 2)
    wk = warr(Wkv[:, :HD], 2)
    wv = warr(Wkv[:, HD:], 2)
    wo = warr(Wo, 4)
    return [
        {"xt": xts[c], "wq": wq, "wk": wk, "wv": wv, "wo": wo} for c in range(CORES)
    ]


def kernel(x, Wq, Wkv, Wo, bo, mask, tie_attn_dim):
    x = np.asarray(x)
    br, n, dim = x.shape
    r = int(tie_attn_dim)
    assert (br, n, dim) == (128, 512, 256) and r == 128, "kernel hardcodes shapes"
    mask = np.asarray(mask)
    assert mask.all(), "kernel assumes an all-valid mask"
    num_rows = float(mask.reshape(1, r, n).any(axis=-1).sum(axis=-1)[0])
    scale = (D ** -0.5) * (num_rows ** -0.5)

    nc = _get_nc(scale)
    in_maps = make_in_maps(x, Wq, Wkv, Wo)
    res = bass_utils.run_bass_kernel_spmd(nc, in_maps, core_ids=list(range(CORES)))
    # outT [2, 128, RN] bf16 dim-major -> [RN, DIM] f32
    out = np.stack(
        [
            np.asarray(m["outT"]).reshape(DIM, RN).T.astype(np.float32)
            for m in res.results
        ]
    )
    out = out.reshape(br, n, dim)
    bo = np.asarray(bo, dtype=np.float32)
    if bo.any():
        out = out + bo
    return np.ascontiguousarray(out.astype(np.float32))
